# revision 34
# baseline (speedup 1.0000x reference)
"""Trainium2 Bass kernel for nn_AttentionOnDetail (dense transformer attention).

Sharding: head-parallel tensor parallelism across 8 NeuronCores.
Each core computes ONE attention head for all 4 batches using its slice of
W_qkvg (column-parallel) and W_out (row-parallel); the host sums the 8
partial outputs (the row-parallel all-reduce, done on host at gather time).

Per-core kernel structure (per batch):
  P1: project q,k TRANSPOSED:  qkT[slot, seq] = W_qk @ x0  (lhsT=W^T, rhs=x0^T)
  P2: project v,g NATURAL:     vg[seq, slot]  = x0 @ W_vg^T (lhsT=x0^T, rhs=W^T)
  RMS norm factors from UN-roped q,k (RoPE is a rotation -> preserves norms):
     sum-of-squares via ones-matmuls on PE; q's factor (with tao0/sqrt(hd))
     multiplied into roped q; k's factor folded into exp()'s per-partition
     scale operand.
  Attention as S^T[k,q] = k~^T q~ (both operands already [hd, seq]); softmax
     without max-subtraction (scores bounded ~23 by RMS norm); denominator
     fused into PV matmul as a 257th all-ones column of v; causal handled by
     lower-triangular block skipping + a tri-mask multiply on diagonal blocks.
  PV: y[q, hd+1] accumulated over k-tiles; epilogue fuses 1/denominator and
     sigmoid(g) gating in one scalar_tensor_tensor; PE-transpose of gated y
     feeds the output projection; partial out rows DMA'd (sink row dropped).
"""

import sys
import os

sys.path.insert(0, "/opt/trn_rl_repo")

import numpy as np
from contextlib import ExitStack
from dataclasses import dataclass

import concourse.bass as bass
import concourse.bacc as bacc
import concourse.tile as tile
from concourse import mybir

F32 = mybir.dt.float32
F32R = mybir.dt.float32r
AF = mybir.ActivationFunctionType
ALU = mybir.AluOpType

N_CORES = 8
N_EMBD = 256
N_HEAD = 8
HEAD_DIM = 256
P = 128  # partitions


@dataclass(frozen=True)
class Cfg:
    nb: int = 4          # batches per core
    t_real: int = 2048   # real tokens (output rows per batch)
    seq: int = 2176      # padded seq (sink + t_real padded to mult of 128)
    reps: int = 1        # repeat whole kernel body (timing-slope builds)
    phase: str = "full"  # debug: p1 | p2 | noexp | full
    # pool sizing knobs
    x0_bufs: int = 2
    qk_bufs: int = 8
    sq_bufs: int = 2
    v_bufs: int = 20
    g_bufs: int = 20
    pt_bufs: int = 3
    psa_bufs: int = 3
    psb_bufs: int = 5

    @property
    def nt(self):
        return self.seq // P


FULL = Cfg()


def _chunks(total, width):
    """[(offset, width), ...] covering `total` in steps of `width`."""
    out = []
    off = 0
    while off < total:
        w = min(width, total - off)
        out.append((off, w))
        off += w
    return out


def build_program(cfg: Cfg, cq2: float, shared_cossin: bool, ck2: float):
    """Build the single-core Bass program (same program runs on all 8 cores;
    per-core differences enter only through input data).

    cq2: (tao0/sqrt(hd))^2 folded into q's norm factor.
    ck2: tao1^2 folded into k's norm factor.
    """
    nc = bacc.Bacc("TRN2", target_bir_lowering=False, debug=False)

    nb, seq, nt = cfg.nb, cfg.seq, cfg.nt
    t_real = cfg.t_real

    # ---- DRAM I/O ----
    x0t = nc.dram_tensor("x0t", [nb, 2, P, seq], F32R, kind="ExternalInput").ap()
    w1t = nc.dram_tensor("w1t", [2, P, 512], F32R, kind="ExternalInput").ap()
    w2t = nc.dram_tensor("w2t", [2, P, 512], F32R, kind="ExternalInput").ap()
    wot = nc.dram_tensor("wot", [2, P, N_EMBD], F32R, kind="ExternalInput").ap()
    cosq = nc.dram_tensor("cosq", [P, seq], F32, kind="ExternalInput").ap()
    sinq = nc.dram_tensor("sinq", [P, seq], F32, kind="ExternalInput").ap()
    if not shared_cossin:
        cosk = nc.dram_tensor("cosk", [P, seq], F32, kind="ExternalInput").ap()
        sink = nc.dram_tensor("sink", [P, seq], F32, kind="ExternalInput").ap()
    trimask = nc.dram_tensor("trimask", [P, P], F32, kind="ExternalInput").ap()
    onesc = nc.dram_tensor("onesc", [P, 2], F32R, kind="ExternalInput").ap()
    ident = nc.dram_tensor("ident", [P, P], F32R, kind="ExternalInput").ap()
    out = nc.dram_tensor("out", [nb, t_real, N_EMBD], F32, kind="ExternalOutput").ap()
    # DRAM scratch for the partition-broadcast of q's norm row
    rq_dram = nc.dram_tensor("rq_scratch", [nb, 1, seq], F32).ap()

    eps = float(np.finfo(np.float32).eps)

    with tile.TileContext(nc) as tc, ExitStack() as ctx:
        consts = ctx.enter_context(tc.tile_pool(name="consts", bufs=1))
        x0p = ctx.enter_context(tc.tile_pool(name="x0", bufs=cfg.x0_bufs))
        qkp = ctx.enter_context(tc.tile_pool(name="qk", bufs=cfg.qk_bufs))
        sqp = ctx.enter_context(tc.tile_pool(name="sq", bufs=cfg.sq_bufs))
        rowp = ctx.enter_context(tc.tile_pool(name="rows", bufs=4))
        rqp = ctx.enter_context(tc.tile_pool(name="rqb", bufs=2))
        vp = ctx.enter_context(tc.tile_pool(name="v", bufs=cfg.v_bufs))
        gp = ctx.enter_context(tc.tile_pool(name="g", bufs=cfg.g_bufs))
        ptp = ctx.enter_context(tc.tile_pool(name="pt", bufs=cfg.pt_bufs))
        yp = ctx.enter_context(tc.tile_pool(name="y", bufs=2))
        ytp = ctx.enter_context(tc.tile_pool(name="yt", bufs=2))
        outp = ctx.enter_context(tc.tile_pool(name="outs", bufs=2))
        smallp = ctx.enter_context(tc.tile_pool(name="small", bufs=6))
        # two PSUM pools: projection-side (A) and attention-side (B), so
        # batch b+1's projections can overlap batch b's attention
        psa = ctx.enter_context(
            tc.tile_pool(name="psa", bufs=cfg.psa_bufs, space="PSUM")
        )
        psb = ctx.enter_context(
            tc.tile_pool(name="psb", bufs=cfg.psb_bufs, space="PSUM")
        )

        def ps_tile(shape, dtype=F32, pool=None):
            p = pool or psa
            return p.tile(shape, dtype, tag="ps", name="ps")

        def rsqrt_dve(dst, u, tmp):
            """dst = 1/sqrt(u), DVE only (no ACT table set needed):
            quake-style int initial guess + 2 Newton iterations."""
            I32 = mybir.dt.int32
            di, ui = dst.bitcast(I32), u.bitcast(I32)
            nc.vector.tensor_scalar(di, ui, 1, None, ALU.logical_shift_right)
            # magic - (i>>1) == (~(i>>1)) + (magic+1); bitwise and arith ops
            # cannot share one tensor_scalar instruction
            nc.vector.tensor_scalar(di, di, -1, None, ALU.bitwise_xor)
            nc.vector.tensor_scalar(di, di, 0x5F3759E0, None, ALU.add)
            for _ in range(2):
                nc.vector.tensor_mul(tmp, dst, dst)       # y^2
                nc.vector.tensor_mul(tmp, tmp, u)         # u*y^2
                nc.vector.tensor_scalar(
                    tmp, tmp, -0.5, 1.5, ALU.mult, ALU.add
                )                                         # 1.5 - u*y^2/2
                nc.vector.tensor_mul(dst, dst, tmp)

        # ---- constants into SBUF ----
        cos_q = consts.tile([P, seq], F32)
        sin_q = consts.tile([P, seq], F32)
        nc.sync.dma_start(cos_q[:], cosq[:])
        nc.sync.dma_start(sin_q[:], sinq[:])
        if shared_cossin:
            cos_k, sin_k = cos_q, sin_q
        else:
            cos_k = consts.tile([P, seq], F32, name="cos_k")
            sin_k = consts.tile([P, seq], F32, name="sin_k")
            nc.sync.dma_start(cos_k[:], cosk[:])
            nc.sync.dma_start(sin_k[:], sink[:])
        w1_sb = [consts.tile([P, 512], F32R, name=f"w1_{e}") for e in range(2)]
        w2_sb = [consts.tile([P, 512], F32R, name=f"w2_{e}") for e in range(2)]
        wo_sb = [consts.tile([P, N_EMBD], F32R, name=f"wo_{e}") for e in range(2)]
        for e in range(2):
            nc.sync.dma_start(w1_sb[e][:], w1t[e])
            nc.sync.dma_start(w2_sb[e][:], w2t[e])
            nc.sync.dma_start(wo_sb[e][:], wot[e])
        tri_sb = consts.tile([P, P], F32)
        id_sb = consts.tile([P, P], F32R)
        nc.sync.dma_start(tri_sb[:], trimask[:])
        nc.sync.dma_start(id_sb[:], ident[:])
        onesP = consts.tile([P, 1], F32R)
        nc.sync.dma_start(onesP[:], onesc[:, 0:1])

        def emit_p1(b):
            """projection of q,k (transposed), norm factors, rope, q-norm."""
            x0_sb = [
                x0p.tile([P, seq], F32R, tag="x0", name=f"x0_{e}")
                for e in range(2)
            ]
            for e in range(2):
                nc.sync.dma_start(x0_sb[e][:], x0t[b, e])

            q1 = qkp.tile([P, seq], F32R, tag="qk", name="q1")
            q2 = qkp.tile([P, seq], F32R, tag="qk", name="q2")
            k1 = qkp.tile([P, seq], F32R, tag="qk", name="k1")
            k2 = qkp.tile([P, seq], F32R, tag="qk", name="k2")

            rq_sb = smallp.tile([P, nt], F32, tag="rq", name="rq")
            rk_sb = smallp.tile([P, nt], F32, tag="rk", name="rk")
            # [P, 4, nt]: separate columns per slot-tile so every matmul is
            # its own single-instruction group (a start=True clears
            # has_written bits bank-wide, so multi-matmul groups must not
            # share a bank with other groups in flight)
            ps_ss = ps_tile([P, 4, nt])

            for coff, cw in _chunks(seq, 512):
                t0 = coff // P
                sl = slice(coff, coff + cw)
                # slot-tiles in (q1,q2) then (k1,k2) pairs to keep the
                # projection-side PSUM footprint at 2 tiles + sumsq
                for pair, (y1, y2, cc, ss) in enumerate((
                    (q1, q2, cos_q, sin_q),
                    (k1, k2, cos_k, sin_k),
                )):
                    ps_c = [ps_tile([P, cw]) for _ in range(2)]
                    for half in range(2):
                        s = 2 * pair + half
                        for e in range(2):
                            nc.tensor.matmul(
                                ps_c[half][:],
                                w1_sb[e][:, s * P : (s + 1) * P],
                                x0_sb[e][:, coff : coff + cw],
                                start=(e == 0),
                                stop=(e == 1),
                            )
                        # squares + per-position sumsq columns (N=1 plain-
                        # fp32 matmuls; f32r codegen rejects N=1)
                        sq_c = sqp.tile([P, cw], F32R, tag="sq", name="sq")
                        nc.scalar.square(sq_c[:], ps_c[half][:])
                        for ti in range(cw // P):
                            nc.tensor.matmul(
                                ps_ss[:, s, t0 + ti : t0 + ti + 1],
                                sq_c[:, ti * P : (ti + 1) * P].bitcast(F32),
                                onesP[:].bitcast(F32),
                                start=True,
                                stop=True,
                            )
                    # rope (raw; q's norm factor applied afterwards)
                    t1 = sqp.tile([P, cw], F32, tag="ropet", name="t1")
                    t2 = sqp.tile([P, cw], F32, tag="ropet", name="t2")
                    nc.vector.tensor_mul(t1[:], ps_c[0][:], cc[:, sl])
                    nc.vector.tensor_mul(t2[:], ps_c[1][:], ss[:, sl])
                    nc.vector.tensor_add(y1[:, sl], t1[:], t2[:])
                    nc.vector.tensor_mul(t1[:], ps_c[0][:], ss[:, sl])
                    nc.vector.tensor_mul(t2[:], ps_c[1][:], cc[:, sl])
                    nc.vector.tensor_sub(y2[:, sl], t2[:], t1[:])

            # norm factors r = |c|*rsqrt(mean+eps) = rsqrt(ms/(hd c^2)+eps/c^2)
            # HW allows at most one PSUM operand per DVE instruction
            for dst, s0, c2 in ((rq_sb, 0, cq2), (rk_sb, 2, ck2)):
                u_t = smallp.tile([P, nt], F32, tag="ut", name="ut")
                nw_t = smallp.tile([P, nt], F32, tag="nwt", name="nwt")
                nc.vector.tensor_copy(u_t[:], ps_ss[:, s0, :])
                nc.vector.tensor_add(u_t[:], u_t[:], ps_ss[:, s0 + 1, :])
                nc.vector.tensor_scalar(
                    u_t[:], u_t[:], 1.0 / (HEAD_DIM * c2), eps / c2,
                    ALU.mult, ALU.add,
                )
                rsqrt_dve(dst[:], u_t[:], nw_t[:])

            # q factor: scatter columns to a DRAM row, broadcast-load back
            # in 512-wide chunks and scale the roped q
            scat = bass.AP(
                tensor=rq_dram.tensor,
                offset=rq_dram[b].offset,
                ap=[[1, P], [P, nt]],
            )
            nc.sync.dma_start(scat, rq_sb[:])
            for coff, cw in _chunks(seq, 512):
                rq_b = rqp.tile([P, 512], F32, tag="rqb", name="rqb")
                bcast = bass.AP(
                    tensor=rq_dram.tensor,
                    offset=rq_dram[b].offset + coff,
                    ap=[[0, P], [1, cw]],
                )
                nc.sync.dma_start(rq_b[:, :cw], bcast)
                sl = slice(coff, coff + cw)
                nc.vector.tensor_mul(q1[:, sl], q1[:, sl], rq_b[:, :cw])
                nc.vector.tensor_mul(q2[:, sl], q2[:, sl], rq_b[:, :cw])

            return dict(b=b, x0=x0_sb, q1=q1, q2=q2, k1=k1, k2=k2, rk=rk_sb)

        def emit_p2(st):
            """v,g projection (natural layout)"""
            x0_sb = st["x0"]
            v_sb, g_sb = [], []
            for t in range(nt):
                ps_vg = ps_tile([P, 512])
                for e in range(2):
                    nc.tensor.matmul(
                        ps_vg[:],
                        x0_sb[e][:, t * P : (t + 1) * P],
                        w2_sb[e][:],
                        start=(e == 0),
                        stop=(e == 1),
                    )
                # 258 = 256 v-cols + TWO ones-cols (f32r needs even N)
                vt = vp.tile([P, 258], F32R, tag="v", name="vt")
                nc.scalar.copy(vt[:, 0:256], ps_vg[:, 0:256])
                nc.sync.dma_start(vt[:, 256:258], onesc[:])
                gt = gp.tile([P, 256], F32, tag="g", name="gt")
                # sigmoid(g) = 0.5*(1+tanh(g/2)); the 0.5 is folded into wot
                nc.scalar.activation(
                    gt[:], ps_vg[:, 256:512], AF.Tanh, scale=0.5
                )
                nc.vector.tensor_scalar(gt[:], gt[:], 1.0, None, ALU.add)
                v_sb.append(vt)
                g_sb.append(gt)
            st["v"], st["g"] = v_sb, g_sb

        def emit_attention(st):
            b = st["b"]
            q1, q2, k1, k2 = st["q1"], st["q2"], st["k1"], st["k2"]
            rk_sb, v_sb, g_sb = st["rk"], st["v"], st["g"]
            # q-chunks of 2 tiles (3 for the tail when nt is odd) so the
            # S matmul moving dim stays >= 256 (f32r full-rate condition)
            qchunks = []
            t0c = 0
            while t0c < nt:
                take = 3 if nt - t0c == 3 else min(2, nt - t0c)
                qchunks.append((t0c, take))
                t0c += take
            # high q-chunks first: v/g tiles for high kt are then consumed
            # early and their slots recycle into the next batch's P2 sooner
            qchunks.reverse()
            for qc0, njt in qchunks:
                qw = njt * P
                qoff = qc0 * P
                ps_y = [ps_tile([P, 258], pool=psb) for _ in range(njt)]
                for kt in range(qc0 + njt):
                    # S at full chunk width (dead sub-blocks cheaper than
                    # narrow f32r matmuls); PV reads live subtiles only
                    j0 = max(0, kt - qc0)
                    ps_s = ps_tile([P, qw], pool=psb)
                    for e, (kk, qq) in enumerate(((k1, q1), (k2, q2))):
                        nc.tensor.matmul(
                            ps_s[:],
                            kk[:, kt * P : (kt + 1) * P],
                            qq[:, qoff : qoff + qw],
                            start=(e == 0),
                            stop=(e == 1),
                        )
                    pt = ptp.tile([P, qw], F32R, tag="pt", name="pt")
                    nc.scalar.activation(
                        pt[:], ps_s[:], AF.Exp, scale=rk_sb[:, kt : kt + 1]
                    )
                    if kt >= qc0:  # diagonal block: causal tri-mask
                        dj = j0 * P
                        nc.vector.tensor_mul(
                            pt[:, dj : dj + P], pt[:, dj : dj + P], tri_sb[:]
                        )
                    for j in range(j0, njt):
                        nc.tensor.matmul(
                            ps_y[j][:],
                            pt[:, j * P : (j + 1) * P],
                            v_sb[kt][:],
                            start=(kt == 0),
                            stop=(kt == qc0 + j),
                        )
                # epilogue per q-subtile
                for j in range(njt):
                    qt = qc0 + j
                    rec = smallp.tile([P, 1], F32, tag="rec", name="rec")
                    nc.vector.reciprocal(rec[:], ps_y[j][:, 256:257])
                    yg = yp.tile([P, 256], F32R, tag="yg", name="yg")
                    nc.vector.scalar_tensor_tensor(
                        yg[:], ps_y[j][:, 0:256], rec[:], g_sb[qt][:],
                        ALU.mult, ALU.mult,
                    )
                    ps_t = ps_tile([P, 256], F32R, pool=psb)
                    ygt = ytp.tile([P, 256], F32R, tag="ygt", name="ygt")
                    for e in range(2):
                        nc.tensor.transpose(
                            ps_t[:, e * P : (e + 1) * P],
                            yg[:, e * P : (e + 1) * P],
                            id_sb[:],
                        )
                    nc.scalar.copy(ygt[:], ps_t[:])
                    ps_o = ps_tile([P, N_EMBD], pool=psb)
                    for e in range(2):
                        nc.tensor.matmul(
                            ps_o[:],
                            ygt[:, e * P : (e + 1) * P],
                            wo_sb[e][:],
                            start=(e == 0),
                            stop=(e == 1),
                        )
                    o_sb = outp.tile([P, N_EMBD], F32, tag="o", name="o_sb")
                    nc.scalar.copy(o_sb[:], ps_o[:])
                    # DMA out, dropping the sink row (seq row 0)
                    r0 = qt * P - 1  # output row of partition 0
                    p0 = 0
                    if qt == 0:
                        r0, p0 = 0, 1
                    rows = min(P - p0, t_real - r0)
                    if rows > 0:
                        nc.sync.dma_start(
                            out[b, r0 : r0 + rows, :], o_sb[p0 : p0 + rows, :]
                        )

        # software-pipelined emission: attention(b-1) sits between P1(b)
        # and P2(b), so batch b's projections overlap batch b-1's attention
        # (v/g buffers are single-batch, so P2(b) follows attention(b-1))
        for rep in range(cfg.reps):
            prev = None
            for b in range(nb):
                st = emit_p1(b)
                if prev is not None and cfg.phase == "full":
                    emit_attention(prev)
                if cfg.phase != "p1":
                    emit_p2(st)
                prev = st
            if prev is not None and cfg.phase == "full":
                emit_attention(prev)

    nc.finalize()
    return nc


def _prep_core_inputs(cfg: Cfg, x, cos, sin, W_qkvg, W_sink, W_out, tao):
    """Host-side shard prep. Returns (shared dict, per-core list of dicts,
    ln_cq, shared_cossin, (sq, sk) signs)."""
    nb, seq, t_real = cfg.nb, cfg.seq, cfg.t_real
    x = np.asarray(x, np.float32)
    cos = np.asarray(cos, np.float32)
    sin = np.asarray(sin, np.float32)
    W_qkvg = np.asarray(W_qkvg, np.float32)
    W_sink = np.asarray(W_sink, np.float32)
    W_out = np.asarray(W_out, np.float32)
    tao = np.asarray(tao, np.float32)

    cq = float(tao[0]) / np.sqrt(HEAD_DIM)
    ck = float(tao[1])
    sq_sign = 1.0 if cq >= 0 else -1.0
    sk_sign = 1.0 if ck >= 0 else -1.0
    cq2 = float(max(cq * cq, 1e-30))
    ck2 = float(max(ck * ck, 1e-30))
    shared_cossin = sq_sign == sk_sign

    # x0^T padded: [nb, 2, 128, seq]
    x0t = np.zeros((nb, 2, P, seq), np.float32)
    for b in range(nb):
        x0 = np.concatenate([W_sink, x[b]], axis=0)  # [t_real+1, emb]
        x0t[b] = _pad_T(x0, seq)

    ct = np.zeros((P, seq), np.float32)
    st = np.zeros((P, seq), np.float32)
    n_pos = min(cos.shape[0], seq)
    ct[:, :n_pos] = cos[:n_pos, 0, :].T
    st[:, :n_pos] = sin[:n_pos, 0, :].T

    shared = {
        "x0t": x0t,
        "cosq": sq_sign * ct,
        "sinq": sq_sign * st,
        "trimask": np.triu(np.ones((P, P), np.float32)),
        "onesc": np.ones((P, 2), np.float32),
        "ident": np.eye(P, dtype=np.float32),
    }
    if not shared_cossin:
        shared["cosk"] = sk_sign * ct
        shared["sink"] = sk_sign * st

    per_core = []
    for h in range(N_CORES):
        r0 = 1024 * h
        w1 = W_qkvg[r0 : r0 + 512].T.copy()          # [256, 512] q|k
        w2 = W_qkvg[r0 + 512 : r0 + 1024].T.copy()   # [256, 512] v|g
        # 0.5 from sigmoid(g) = 0.5*(1+tanh(g/2)) folded into the out proj
        wo = 0.5 * W_out[:, 256 * h : 256 * (h + 1)].T  # [256 hd, 256 emb]
        per_core.append(
            {
                "w1t": w1.reshape(2, P, 512),
                "w2t": w2.reshape(2, P, 512),
                "wot": wo.reshape(2, P, N_EMBD),
            }
        )
    return shared, per_core, cq2, shared_cossin, ck2


def _pad_T(x0, seq):
    """[rows, 256] -> transposed+padded [2, 128, seq]"""
    buf = np.zeros((N_EMBD, seq), np.float32)
    buf[:, : x0.shape[0]] = x0.T
    return buf.reshape(2, P, seq)


def kernel(x, cos, sin, W_qkvg, W_sink, W_out, tao, n_head):
    assert int(n_head) == N_HEAD
    cfg = FULL
    shared, per_core, cq2, shared_cossin, ck2 = _prep_core_inputs(
        cfg, x, cos, sin, W_qkvg, W_sink, W_out, tao
    )
    nc = build_program_cached(cfg, cq2, shared_cossin, ck2)

    in_maps = [dict(shared, **pc) for pc in per_core]
    from concourse.bass_utils import run_bass_kernel_spmd

    res = run_bass_kernel_spmd(nc, in_maps, core_ids=list(range(N_CORES)))
    total = np.zeros((cfg.nb, cfg.t_real, N_EMBD), np.float32)
    for r in res.results:
        total += r["out"]
    return total


_PROGRAM_CACHE = {}


def build_program_cached(cfg, cq2, shared_cossin, ck2):
    key = (cfg, round(cq2, 14), shared_cossin, round(ck2, 14))
    if key not in _PROGRAM_CACHE:
        _PROGRAM_CACHE[key] = build_program(cfg, cq2, shared_cossin, ck2)
    return _PROGRAM_CACHE[key]
